# revision 19
# baseline (speedup 1.0000x reference)
"""Causal self-attention Trainium2 kernel.

B=2, T=2048, C=1024, H=16, D=64, 8 NeuronCores.
Sharding: core i handles batch b=i//4 and heads [4*(i%4), 4*(i%4)+4).
Host transposes x[b] -> xT, slices w_qkv/w_proj per core, and sums the 4
per-batch partial output projections at the end.

All matmuls run in float32r (TF32-like, 1 cyc/row at N>=256).
Scores are computed transposed (S^T[j,i]) so softmax exp/mask are free-dim
ops and P^T feeds the attention*V matmul as the moving operand. A ones
column appended to V yields the softmax denominator for free.
"""

import numpy as np
from contextlib import ExitStack

import concourse.bacc as bacc
import concourse.mybir as mybir
import concourse.tile as tile
from concourse.bass_utils import run_bass_kernel_spmd

B, T, C = 2, 2048, 1024
N_HEAD = 16
D = C // N_HEAD  # 64
N_CORES = 8
HPC = 4  # heads per core
TB = T // 512  # 4 i-blocks of 512
NJ = T // 128  # 16 j-chunks of 128

F32 = mybir.dt.float32
F32R = mybir.dt.float32r

_compiled = None


def _build_mask():
    """M[j, x] = 1.0 iff (x - 384) >= j, shape [128, 896].

    For diagonal-region tile (jc, ib) with r = jc - 4*ib in {0..3}, the
    0/1 mask over [128 j, 512 i'] is M[:, 384-128r : 896-128r].
    """
    j = np.arange(128)[:, None]
    x = np.arange(896)[None, :]
    return ((x - 384) >= j).astype(np.float32)


def _build_nc():
    nc = bacc.Bacc("TRN2", target_bir_lowering=False, debug=False,
                   num_devices=N_CORES)

    xt_t = nc.dram_tensor("xt", [C, T], F32R, kind="ExternalInput")
    wqk_t = nc.dram_tensor("wqk", [C, 8 * D], F32R, kind="ExternalInput")
    wv_t = nc.dram_tensor("wv", [C, 4 * D], F32R, kind="ExternalInput")
    wp_t = nc.dram_tensor("wp", [4 * D, C], F32R, kind="ExternalInput")
    mask_t = nc.dram_tensor("mask", [128, 896], F32R, kind="ExternalInput")
    vinit_t = nc.dram_tensor("vinit", [128, 128 * NJ], F32R, kind="ExternalInput")
    zinit_t = nc.dram_tensor("zinit", [64, T], F32R, kind="ExternalInput")
    sel_t = nc.dram_tensor("sel", [8, 8 * 64], F32R, kind="ExternalInput")
    out_t = nc.dram_tensor("out", [T, C], F32, kind="ExternalOutput")

    Exp = mybir.ActivationFunctionType.Exp
    Ln = mybir.ActivationFunctionType.Ln

    with tile.TileContext(nc) as tc, ExitStack() as ctx:
        sb = ctx.enter_context(tc.tile_pool(name="sb", bufs=1))
        ps = ctx.enter_context(tc.tile_pool(name="ps", bufs=4, space="PSUM"))
        ps_y = ctx.enter_context(tc.tile_pool(name="psy", bufs=2, space="PSUM"))
        ps_b = ctx.enter_context(tc.tile_pool(name="psb", bufs=2, space="PSUM"))

        # ---- persistent SBUF (pool sb) ----
        mask_s = sb.tile([128, 896], F32R, tag="mask")
        sel_s = sb.tile([8, 8 * 64], F32R, tag="sel")
        wp_s = [sb.tile([128, C], F32R, tag=f"wp{p}", name=f"wp{p}")
                for p in range(2)]
        # qT pair tiles: partitions 0-63 head 2p, 64-127 head 2p+1
        qT = [sb.tile([128, T], F32R, tag=f"qT{p}", name=f"qT{p}")
              for p in range(2)]
        # kT per head, full 128 partitions: the head's own 64 rows hold k,
        # the complementary 64 rows are zero so S^T matmuls run K=128
        # (full-array geometry keeps the PE HAM clock-gate warm)
        kT = [sb.tile([128, T], F32R, tag=f"kT{h}", name=f"kT{h}")
              for h in range(HPC)]
        # v per head: [128 t-part, 128*NJ]; per 128-chunk: cols 0-63 = v,
        # col 64 = ones (softmax denominator), cols 65-127 = ones (padding
        # so AV matmuls run M=128 full-geometry; output rows 65-127 unused)
        v_s = [sb.tile([128, 128 * NJ], F32R, tag=f"v{h}", name=f"v{h}")
               for h in range(HPC)]
        # yT pair tiles: partitions 0-63 head 2p, 64-127 head 2p+1
        yT = [sb.tile([128, T], F32R, tag=f"yT{p}", name=f"yT{p}")
              for p in range(2)]

        # ---- phase A: qkv projections (pool pa freed afterwards) ----
        # xt loads split into 512-col quarters, consumed nb-outer, so the
        # first matmul chain starts after ~2MB of DMA instead of ~11MB
        with tc.tile_pool(name="pa", bufs=1) as pa:
            wqk_s = [pa.tile([128, 8 * D], F32R, tag=f"wqk{kc}",
                             name=f"wqk{kc}") for kc in range(8)]
            for kc in range(8):
                nc.sync.dma_start(wqk_s[kc][:],
                                  wqk_t.ap()[128 * kc:128 * (kc + 1), :])
            xt_s = {}
            for nb in range(TB):
                for kc in range(8):
                    t = pa.tile([128, 512], F32R, tag=f"xt{kc}_{nb}",
                                name=f"xt{kc}_{nb}")
                    nc.sync.dma_start(
                        t[:], xt_t.ap()[128 * kc:128 * (kc + 1),
                                        512 * nb:512 * (nb + 1)])
                    xt_s[(kc, nb)] = t
            wv_s = [pa.tile([128, 4 * D], F32R, tag=f"wv{kc}",
                            name=f"wv{kc}") for kc in range(8)]
            for kc in range(8):
                nc.sync.dma_start(wv_s[kc][:],
                                  wv_t.ap()[128 * kc:128 * (kc + 1), :])
            # deferred constant loads (phase B/C only) on the scalar queue
            for h in range(HPC):
                po = 64 * (h % 2)
                nc.scalar.dma_start(kT[h][64 - po:128 - po, :],
                                    zinit_t.ap()[:])
            for h in range(HPC):
                nc.scalar.dma_start(v_s[h][:], vinit_t.ap()[:])
            nc.scalar.dma_start(mask_s[:], mask_t.ap()[:])
            nc.scalar.dma_start(sel_s[:], sel_t.ap()[:])
            for p in range(2):
                nc.scalar.dma_start(wp_s[p][:],
                                    wp_t.ap()[128 * p:128 * (p + 1), :])
            for nb in range(TB):
                # qT/kT: mc 0,1 q pairs; 2,3 k pairs
                for mc in range(4):
                    p = ps.tile([128, 512], F32, tag="mm")
                    for kc in range(8):
                        nc.tensor.matmul(
                            p[:], wqk_s[kc][:, 128 * mc:128 * (mc + 1)],
                            xt_s[(kc, nb)][:],
                            start=(kc == 0), stop=(kc == 7))
                    if mc < 2:
                        nc.vector.tensor_copy(
                            qT[mc][:, 512 * nb:512 * (nb + 1)], p[:])
                    else:
                        for s in range(2):
                            h = 2 * (mc - 2) + s
                            nc.vector.tensor_copy(
                                kT[h][64 * s:64 * (s + 1),
                                      512 * nb:512 * (nb + 1)],
                                p[64 * s:64 * (s + 1), :])
                # v: out[tc] = xT[:, tc].T @ wv  -> [128 t, 256]
                for tq in range(4):
                    tci = 4 * nb + tq
                    p = ps.tile([128, 256], F32, tag="mm")
                    for kc in range(8):
                        nc.tensor.matmul(
                            p[:],
                            xt_s[(kc, nb)][:, 128 * tq:128 * (tq + 1)],
                            wv_s[kc][:], start=(kc == 0), stop=(kc == 7))
                    for h in range(HPC):
                        nc.vector.tensor_copy(
                            v_s[h][:, 128 * tci:128 * tci + 64],
                            p[:, 64 * h:64 * (h + 1)])

        with tc.tile_pool(name="pb", bufs=1) as pb:
            # ---- phase B: attention per head; PE stream is pure S/AV ----
            # (normalization deferred so the PE never stalls: PE idle gaps
            # >3.4us trip the HAM clock-gate re-throttle)
            drows = [pb.tile([8, 512], F32, tag=f"drows{i}",
                             name=f"drows{i}") for i in range(2)]
            # unnormalized y staging for odd heads (normalized then moved
            # to yT rows 64-127)
            ytmp_all = pb.tile([64, 4096], F32R, tag="ytmp_all")

            def emit_norm_half(half):
                # batch reciprocal of 8 denominator rows: 1/d = exp(-ln d),
                # then per-block broadcast via selector matmul + normalize
                lnd = pb.tile([8, 512], F32, tag=f"lnd{half}",
                              name=f"lnd{half}")
                nc.scalar.activation(lnd[:], drows[half][:], Ln)
                recs = pb.tile([8, 512], F32R, tag=f"recs{half}",
                               name=f"recs{half}")
                nc.scalar.activation(recs[:], lnd[:], Exp, scale=-1.0)
                for b8 in range(8):
                    blk = 8 * half + b8
                    h, ib = blk // 4, blk % 4
                    prf = ps_b.tile([64, 512], F32, tag="bcast")
                    nc.tensor.matmul(prf[:],
                                     sel_s[:, 64 * b8:64 * (b8 + 1)],
                                     recs[:], start=True, stop=True)
                    if h % 2 == 0:
                        nc.vector.tensor_mul(
                            yT[h // 2][0:64, 512 * ib:512 * (ib + 1)],
                            yT[h // 2][0:64, 512 * ib:512 * (ib + 1)],
                            prf[:])
                    else:
                        oidx = 4 * (h // 2) + ib
                        yn = pb.tile([64, 512], F32R, tag="yn", bufs=2,
                                     name="yn")
                        nc.vector.tensor_mul(
                            yn[:],
                            ytmp_all[:, 512 * oidx:512 * (oidx + 1)],
                            prf[:])
                        nc.scalar.dma_start(
                            yT[h // 2][64:128, 512 * ib:512 * (ib + 1)],
                            yn[:])

            for h in range(HPC):
                qTt = qT[h // 2]
                for ib in range(TB):
                    blk = 4 * h + ib
                    jhi = 4 * ib + 3
                    py = ps_y.tile([128, 512], F32, tag="avy")
                    avq = []

                    def emit_av(ent, py=py, h=h, jhi=jhi):
                        jc, pt, off, w = ent
                        nc.tensor.matmul(
                            py[:, off:512],
                            v_s[h][:, 128 * jc:128 * (jc + 1)],
                            pt[:, 0:w], start=(jc == 0), stop=(jc == jhi))

                    for jc in range(jhi + 1):
                        r = jc - 4 * ib
                        off = 128 * r if r > 0 else 0
                        w = 512 - off
                        p_s = ps.tile([128, 512], F32, tag="mm")
                        nc.tensor.matmul(
                            p_s[:, 0:w],
                            kT[h][:, 128 * jc:128 * (jc + 1)],
                            qTt[:, 512 * ib + off:512 * (ib + 1)],
                            start=True, stop=True)
                        pt = pb.tile([128, 512], F32R, tag="P", bufs=4,
                                     name="pt")
                        nc.scalar.activation(pt[:, 0:w], p_s[:, 0:w], Exp,
                                             scale=0.125)
                        if r >= 0:
                            # triangular sub-block = first 128 trimmed cols
                            nc.gpsimd.tensor_mul(
                                pt[:, 0:128], pt[:, 0:128],
                                mask_s[:, 384:512])
                        avq.append((jc, pt, off, w))
                        if jc >= 2:
                            emit_av(avq.pop(0))
                    while avq:
                        emit_av(avq.pop(0))
                    # extract denominator row + unnormalized y (DVE+DMA
                    # only; ACT stays on exp, PE untouched)
                    dtmp = pb.tile([1, 512], F32, tag="dtmp", bufs=2)
                    nc.vector.tensor_copy(dtmp[:], py[64:65, :])
                    nc.sync.dma_start(drows[blk // 8][blk % 8:blk % 8 + 1, :],
                                      dtmp[:])
                    if h % 2 == 0:
                        nc.vector.tensor_copy(
                            yT[h // 2][0:64, 512 * ib:512 * (ib + 1)],
                            py[0:64, :])
                    else:
                        oidx = 4 * (h // 2) + ib
                        nc.vector.tensor_copy(
                            ytmp_all[:, 512 * oidx:512 * (oidx + 1)],
                            py[0:64, :])
                    if blk == 7:
                        # first half's normalization weaves into the
                        # second half's S/AV stream
                        emit_norm_half(0)
            emit_norm_half(1)

            # ---- phase C: output projection (K=128 per head pair) ----
            with tc.tile_pool(name="pc", bufs=1) as pc:
                for tb in range(NJ):
                    for n in range(2):
                        p = ps.tile([128, 512], F32, tag="mm")
                        for pp in range(2):
                            nc.tensor.matmul(
                                p[:], yT[pp][:, 128 * tb:128 * (tb + 1)],
                                wp_s[pp][:, 512 * n:512 * (n + 1)],
                                start=(pp == 0), stop=(pp == 1))
                        o = pc.tile([128, 512], F32, tag="o", bufs=2)
                        nc.vector.tensor_copy(o[:], p[:])
                        nc.sync.dma_start(
                            out_t.ap()[128 * tb:128 * (tb + 1),
                                       512 * n:512 * (n + 1)], o[:])

    nc.compile()
    return nc


def _get_compiled():
    global _compiled
    if _compiled is None:
        _compiled = _build_nc()
    return _compiled


def _in_maps(x, w_qkv, w_proj):
    x = np.asarray(x, dtype=np.float32)
    w_qkv = np.asarray(w_qkv, dtype=np.float32)
    w_proj = np.asarray(w_proj, dtype=np.float32)
    mask = _build_mask()
    sel = np.zeros((8, 8 * 64), dtype=np.float32)
    for b in range(8):
        sel[b, 64 * b:64 * (b + 1)] = 1.0
    maps = []
    for core in range(N_CORES):
        b = core // 4
        h0 = 4 * (core % 4)
        heads = range(h0, h0 + HPC)
        xt = np.ascontiguousarray(x[b].T)  # [C, T]
        wqk = np.concatenate(
            [w_qkv[:, 64 * h:64 * (h + 1)] for h in heads]
            + [w_qkv[:, C + 64 * h:C + 64 * (h + 1)] for h in heads], axis=1)
        wv = np.concatenate(
            [w_qkv[:, 2 * C + 64 * h:2 * C + 64 * (h + 1)] for h in heads],
            axis=1)
        wp = np.concatenate(
            [w_proj[64 * h:64 * (h + 1), :] for h in heads], axis=0)
        maps.append({
            "xt": np.ascontiguousarray(xt),
            "wqk": np.ascontiguousarray(wqk),
            "wv": np.ascontiguousarray(wv),
            "wp": np.ascontiguousarray(wp),
            "mask": mask,
            "vinit": np.ones((128, 128 * NJ), dtype=np.float32),
            "zinit": np.zeros((64, T), dtype=np.float32),
            "sel": sel,
        })
    return maps


def _combine(results, b_proj):
    out = np.zeros((B, T, C), dtype=np.float32)
    for core in range(N_CORES):
        out[core // 4] += results[core]["out"]
    out += np.asarray(b_proj, dtype=np.float32)[None, None, :]
    return out


def kernel(x, w_qkv, w_proj, b_proj):
    nc = _get_compiled()
    res = run_bass_kernel_spmd(nc, _in_maps(x, w_qkv, w_proj),
                               core_ids=list(range(N_CORES)))
    return _combine(res.results, b_proj)


def kernel_traced(x, w_qkv, w_proj, b_proj):
    """Like kernel() but with NTFF tracing; returns (out, BassKernelResults)."""
    nc = _get_compiled()
    res = run_bass_kernel_spmd(nc, _in_maps(x, w_qkv, w_proj),
                               core_ids=list(range(N_CORES)), trace=True)
    return _combine(res.results, b_proj), res


# revision 20
# speedup vs baseline: 1.1226x; 1.1226x over previous
"""Causal self-attention Trainium2 kernel.

B=2, T=2048, C=1024, H=16, D=64, 8 NeuronCores.
Sharding: core i handles batch b=i//4 and heads [4*(i%4), 4*(i%4)+4).
Host transposes x[b] -> xT, slices w_qkv/w_proj per core, and sums the 4
per-batch partial output projections at the end.

All matmuls run in float32r (TF32-like, 1 cyc/row at N>=256).
Scores are computed transposed (S^T[j,i]) so softmax exp/mask are free-dim
ops and P^T feeds the attention*V matmul as the moving operand. A ones
column appended to V yields the softmax denominator for free.
"""

import numpy as np
from contextlib import ExitStack

import concourse.bacc as bacc
import concourse.mybir as mybir
import concourse.tile as tile
from concourse.bass_utils import run_bass_kernel_spmd

B, T, C = 2, 2048, 1024
N_HEAD = 16
D = C // N_HEAD  # 64
N_CORES = 8
HPC = 4  # heads per core
TB = T // 512  # 4 i-blocks of 512
NJ = T // 128  # 16 j-chunks of 128

F32 = mybir.dt.float32
F32R = mybir.dt.float32r

_compiled = None


def _build_mask():
    """M[j, x] = 1.0 iff (x - 384) >= j, shape [128, 896].

    For diagonal-region tile (jc, ib) with r = jc - 4*ib in {0..3}, the
    0/1 mask over [128 j, 512 i'] is M[:, 384-128r : 896-128r].
    """
    j = np.arange(128)[:, None]
    x = np.arange(896)[None, :]
    return ((x - 384) >= j).astype(np.float32)


def _build_nc():
    nc = bacc.Bacc("TRN2", target_bir_lowering=False, debug=False,
                   num_devices=N_CORES)

    xt_t = nc.dram_tensor("xt", [C, T], F32R, kind="ExternalInput")
    wqk_t = nc.dram_tensor("wqk", [C, 8 * D], F32R, kind="ExternalInput")
    wv_t = nc.dram_tensor("wv", [C, 4 * D], F32R, kind="ExternalInput")
    wp_t = nc.dram_tensor("wp", [4 * D, C], F32R, kind="ExternalInput")
    mask_t = nc.dram_tensor("mask", [128, 896], F32R, kind="ExternalInput")
    vinit_t = nc.dram_tensor("vinit", [128, 64 * NJ], F32R, kind="ExternalInput")
    zinit_t = nc.dram_tensor("zinit", [64, T], F32R, kind="ExternalInput")
    sel_t = nc.dram_tensor("sel", [4, 4 * 64], F32R, kind="ExternalInput")
    out_t = nc.dram_tensor("out", [T, C], F32, kind="ExternalOutput")

    Exp = mybir.ActivationFunctionType.Exp
    Ln = mybir.ActivationFunctionType.Ln

    with tile.TileContext(nc) as tc, ExitStack() as ctx:
        sb = ctx.enter_context(tc.tile_pool(name="sb", bufs=1))
        ps = ctx.enter_context(tc.tile_pool(name="ps", bufs=4, space="PSUM"))
        ps_y = ctx.enter_context(tc.tile_pool(name="psy", bufs=2, space="PSUM"))
        ps_b = ctx.enter_context(tc.tile_pool(name="psb", bufs=2, space="PSUM"))

        # ---- persistent SBUF (pool sb) ----
        mask_s = sb.tile([128, 896], F32R, tag="mask")
        sel_s = sb.tile([4, 4 * 64], F32R, tag="sel")
        wp_s = [sb.tile([128, C], F32R, tag=f"wp{p}", name=f"wp{p}")
                for p in range(2)]
        qT = [sb.tile([128, T], F32R, tag=f"qT{p}", name=f"qT{p}")
              for p in range(2)]
        # kT per head, full 128 partitions: the head's own 64 rows hold k,
        # the complementary 64 rows are zero so S^T matmuls run K=128
        # (full-array geometry keeps the PE HAM clock-gate warm)
        kT = [sb.tile([128, T], F32R, tag=f"kT{h}", name=f"kT{h}")
              for h in range(HPC)]
        # v per head: [128 t-part, 128*NJ]; per 128-chunk: cols 0-63 = v,
        # col 64 = ones (softmax denominator), cols 65-127 = ones (padding
        # so AV matmuls run M=128 full-geometry; output rows 65-127 unused)
        v_s = [sb.tile([128, 128 * NJ], F32R, tag=f"v{h}", name=f"v{h}")
               for h in range(HPC)]
        yT = [sb.tile([128, T], F32R, tag=f"yT{p}", name=f"yT{p}")
              for p in range(2)]

        # ---- phase A: qkv projections (pool pa freed afterwards) ----
        # DMA order matters: wqk + xt nb0 + wv + vinit arrive first so the
        # matmul stream starts ~12us in; phase-B/C constants are pushed
        # behind via tile_wait_until so they don't steal HBM bandwidth.
        with tc.tile_pool(name="pa", bufs=1) as pa:
            wqk_s = [pa.tile([128, 8 * D], F32R, tag=f"wqk{kc}",
                             name=f"wqk{kc}") for kc in range(8)]
            for kc in range(8):
                nc.sync.dma_start(wqk_s[kc][:],
                                  wqk_t.ap()[128 * kc:128 * (kc + 1), :])
            xt_s = {}
            for nb in range(TB):
                for kc in range(8):
                    t = pa.tile([128, 512], F32R, tag=f"xt{kc}_{nb}",
                                name=f"xt{kc}_{nb}")
                    nc.sync.dma_start(
                        t[:], xt_t.ap()[128 * kc:128 * (kc + 1),
                                        512 * nb:512 * (nb + 1)])
                    xt_s[(kc, nb)] = t
                if nb == 0:
                    wv_s = [pa.tile([128, 4 * D], F32R, tag=f"wv{kc}",
                                    name=f"wv{kc}") for kc in range(8)]
                    for kc in range(8):
                        nc.sync.dma_start(
                            wv_s[kc][:],
                            wv_t.ap()[128 * kc:128 * (kc + 1), :])
                    # v ones-padding: cols 64-127 of each 128-block
                    for h in range(HPC):
                        dst = v_s[h][:].rearrange("p (c w) -> p c w",
                                                  w=128)[:, :, 64:128]
                        nc.scalar.dma_start(
                            dst, vinit_t.ap()[:].rearrange(
                                "p (c w) -> p c w", w=64))
            with tc.tile_wait_until(0.04):
                for h in range(HPC):
                    po = 64 * (h % 2)
                    nc.scalar.dma_start(kT[h][64 - po:128 - po, :],
                                        zinit_t.ap()[:])
                nc.scalar.dma_start(mask_s[:], mask_t.ap()[:])
                nc.scalar.dma_start(sel_s[:], sel_t.ap()[:])
                for p in range(2):
                    nc.scalar.dma_start(wp_s[p][:],
                                        wp_t.ap()[128 * p:128 * (p + 1), :])
            for nb in range(TB):
                # qT/kT: mc 0,1 q pairs; 2,3 k pairs
                for mc in range(4):
                    p = ps.tile([128, 512], F32, tag="mm")
                    for kc in range(8):
                        nc.tensor.matmul(
                            p[:], wqk_s[kc][:, 128 * mc:128 * (mc + 1)],
                            xt_s[(kc, nb)][:],
                            start=(kc == 0), stop=(kc == 7))
                    if mc < 2:
                        nc.vector.tensor_copy(
                            qT[mc][:, 512 * nb:512 * (nb + 1)], p[:])
                    else:
                        for s in range(2):
                            h = 2 * (mc - 2) + s
                            nc.vector.tensor_copy(
                                kT[h][64 * s:64 * (s + 1),
                                      512 * nb:512 * (nb + 1)],
                                p[64 * s:64 * (s + 1), :])
                # v: out[tc] = xT[:, tc].T @ wv  -> [128 t, 256]
                for tq in range(4):
                    tci = 4 * nb + tq
                    p = ps.tile([128, 256], F32, tag="mm")
                    for kc in range(8):
                        nc.tensor.matmul(
                            p[:],
                            xt_s[(kc, nb)][:, 128 * tq:128 * (tq + 1)],
                            wv_s[kc][:], start=(kc == 0), stop=(kc == 7))
                    for h in range(HPC):
                        nc.vector.tensor_copy(
                            v_s[h][:, 128 * tci:128 * tci + 64],
                            p[:, 64 * h:64 * (h + 1)])

        with tc.tile_pool(name="pb", bufs=1) as pb:
            # ---- phase B: attention; pure S/AV on the PE ----
            # (normalization + output projection for i-block ib weave into
            # the stream one block after head 3 finishes ib, so the PE
            # never idles: idle gaps >3.4us trip the HAM re-throttle)
            drows = [pb.tile([4, 512], F32, tag=f"dr{ib}", name=f"dr{ib}")
                     for ib in range(TB)]
            # unnormalized y staging for odd heads (normalized then moved
            # to yT rows 64-127)
            ytmp_all = pb.tile([64, 4096], F32R, tag="ytmp_all")

            def make_normproj(ib):
                def fin():
                    # reciprocals of the 4 heads' denom rows for this ib
                    lnd = pb.tile([4, 512], F32, tag="lnd", bufs=2,
                                  name="lnd")
                    nc.scalar.activation(lnd[:], drows[ib][:], Ln)
                    recs = pb.tile([4, 512], F32R, tag="recs", bufs=2,
                                   name="recs")
                    nc.scalar.activation(recs[:], lnd[:], Exp, scale=-1.0)
                    for h in range(HPC):
                        prf = ps_b.tile([64, 512], F32, tag="bcast")
                        nc.tensor.matmul(prf[:],
                                         sel_s[:, 64 * h:64 * (h + 1)],
                                         recs[:], start=True, stop=True)
                        if h % 2 == 0:
                            nc.vector.tensor_mul(
                                yT[h // 2][0:64, 512 * ib:512 * (ib + 1)],
                                yT[h // 2][0:64, 512 * ib:512 * (ib + 1)],
                                prf[:])
                        else:
                            oidx = 4 * (h // 2) + ib
                            yn = pb.tile([64, 512], F32R, tag="yn", bufs=2,
                                         name="yn")
                            nc.vector.tensor_mul(
                                yn[:],
                                ytmp_all[:, 512 * oidx:512 * (oidx + 1)],
                                prf[:])
                            nc.scalar.dma_start(
                                yT[h // 2][64:128,
                                           512 * ib:512 * (ib + 1)], yn[:])
                    # output projection for the 4 t-chunks of this ib
                    for tb in range(4 * ib, 4 * ib + 4):
                        for n in range(2):
                            p = ps.tile([128, 512], F32, tag="mm")
                            for pp in range(2):
                                nc.tensor.matmul(
                                    p[:],
                                    yT[pp][:, 128 * tb:128 * (tb + 1)],
                                    wp_s[pp][:, 512 * n:512 * (n + 1)],
                                    start=(pp == 0), stop=(pp == 1))
                            o = pb.tile([128, 512], F32, tag="o", bufs=2,
                                        name="o")
                            nc.vector.tensor_copy(o[:], p[:])
                            nc.sync.dma_start(
                                out_t.ap()[128 * tb:128 * (tb + 1),
                                           512 * n:512 * (n + 1)], o[:])
                return fin

            pending = None
            for h in range(HPC):
                qTt = qT[h // 2]
                for ib in range(TB):
                    jhi = 4 * ib + 3
                    py = ps_y.tile([128, 512], F32, tag="avy")
                    avq = []

                    def emit_av(ent, py=py, h=h, jhi=jhi):
                        jc, pt, off, w = ent
                        nc.tensor.matmul(
                            py[:, off:512],
                            v_s[h][:, 128 * jc:128 * (jc + 1)],
                            pt[:, 0:w], start=(jc == 0), stop=(jc == jhi))

                    for jc in range(jhi + 1):
                        r = jc - 4 * ib
                        off = 128 * r if r > 0 else 0
                        w = 512 - off
                        p_s = ps.tile([128, 512], F32, tag="mm")
                        nc.tensor.matmul(
                            p_s[:, 0:w],
                            kT[h][:, 128 * jc:128 * (jc + 1)],
                            qTt[:, 512 * ib + off:512 * (ib + 1)],
                            start=True, stop=True)
                        pt = pb.tile([128, 512], F32R, tag="P", bufs=4,
                                     name="pt")
                        nc.scalar.activation(pt[:, 0:w], p_s[:, 0:w], Exp,
                                             scale=0.125)
                        if r >= 0:
                            # triangular sub-block = first 128 trimmed cols
                            nc.gpsimd.tensor_mul(
                                pt[:, 0:128], pt[:, 0:128],
                                mask_s[:, 384:512])
                        avq.append((jc, pt, off, w))
                        if jc == 3 and pending is not None:
                            pending()
                            pending = None
                        if jc >= 2:
                            emit_av(avq.pop(0))
                    while avq:
                        emit_av(avq.pop(0))
                    # extract denominator row + unnormalized y (DVE+DMA
                    # only; ACT stays on exp, PE untouched)
                    dtmp = pb.tile([1, 512], F32, tag="dtmp", bufs=2)
                    nc.vector.tensor_copy(dtmp[:], py[64:65, :])
                    nc.sync.dma_start(drows[ib][h:h + 1, :], dtmp[:])
                    if h % 2 == 0:
                        nc.vector.tensor_copy(
                            yT[h // 2][0:64, 512 * ib:512 * (ib + 1)],
                            py[0:64, :])
                    else:
                        oidx = 4 * (h // 2) + ib
                        nc.vector.tensor_copy(
                            ytmp_all[:, 512 * oidx:512 * (oidx + 1)],
                            py[0:64, :])
                    if h == HPC - 1:
                        pending = make_normproj(ib)
            pending()

    nc.compile()
    return nc


def _get_compiled():
    global _compiled
    if _compiled is None:
        _compiled = _build_nc()
    return _compiled


def _in_maps(x, w_qkv, w_proj):
    x = np.asarray(x, dtype=np.float32)
    w_qkv = np.asarray(w_qkv, dtype=np.float32)
    w_proj = np.asarray(w_proj, dtype=np.float32)
    mask = _build_mask()
    sel = np.zeros((4, 4 * 64), dtype=np.float32)
    for b in range(4):
        sel[b, 64 * b:64 * (b + 1)] = 1.0
    maps = []
    for core in range(N_CORES):
        b = core // 4
        h0 = 4 * (core % 4)
        heads = range(h0, h0 + HPC)
        xt = np.ascontiguousarray(x[b].T)  # [C, T]
        wqk = np.concatenate(
            [w_qkv[:, 64 * h:64 * (h + 1)] for h in heads]
            + [w_qkv[:, C + 64 * h:C + 64 * (h + 1)] for h in heads], axis=1)
        wv = np.concatenate(
            [w_qkv[:, 2 * C + 64 * h:2 * C + 64 * (h + 1)] for h in heads],
            axis=1)
        wp = np.concatenate(
            [w_proj[64 * h:64 * (h + 1), :] for h in heads], axis=0)
        maps.append({
            "xt": np.ascontiguousarray(xt),
            "wqk": np.ascontiguousarray(wqk),
            "wv": np.ascontiguousarray(wv),
            "wp": np.ascontiguousarray(wp),
            "mask": mask,
            "vinit": np.ones((128, 64 * NJ), dtype=np.float32),
            "zinit": np.zeros((64, T), dtype=np.float32),
            "sel": sel,
        })
    return maps


def _combine(results, b_proj):
    out = np.zeros((B, T, C), dtype=np.float32)
    for core in range(N_CORES):
        out[core // 4] += results[core]["out"]
    out += np.asarray(b_proj, dtype=np.float32)[None, None, :]
    return out


def kernel(x, w_qkv, w_proj, b_proj):
    nc = _get_compiled()
    res = run_bass_kernel_spmd(nc, _in_maps(x, w_qkv, w_proj),
                               core_ids=list(range(N_CORES)))
    return _combine(res.results, b_proj)


def kernel_traced(x, w_qkv, w_proj, b_proj):
    """Like kernel() but with NTFF tracing; returns (out, BassKernelResults)."""
    nc = _get_compiled()
    res = run_bass_kernel_spmd(nc, _in_maps(x, w_qkv, w_proj),
                               core_ids=list(range(N_CORES)), trace=True)
    return _combine(res.results, b_proj), res
